# revision 4
# baseline (speedup 1.0000x reference)
"""Trainium2 Bass kernel for the stacked tanh-RNN (BredictNetwork).

Strategy: data-parallel over batch (4096 -> 512/core on 8 cores). Per core a
*wavefront* schedule updates all 8 layers simultaneously each time step:
  H_s[l] = tanh(Wih_l @ H_{s-1}[l-1] + Whh_l @ H_{s-1}[l] + b_l),  H_s[-1]=x_s
State layout [128 part, 128 free]: partition p = l*16 + g*4 + h (l layer,
g batch-group of 128, h hidden), free = batch-within-group. One 128x128
fp16 matmul (block-structured weights, host-assembled, fp32 PSUM) + one
PSUM-accumulate inject matmul (the x drive, from a DMA'd ring of relayouted
x) + one tanh (per-partition fp32 bias) per step; 2048+7 steps. FC head
computed on-core at the end. h_n captured from the last 8 wavefront steps.
fp16 operand precision validated offline: out relerr 1.5e-4, hn 9.7e-4.
"""
import os
import numpy as np

T = 2048
L = 8
HID = 4
B = 4096
NCORES = 8
BC = B // NCORES          # 512 batch per core
NSTEPS = T + L - 1        # 2055
NCHUNK = T // 8           # 256 ring chunks (8 steps each)
CCOLS = 1280              # fp16 const block columns (padded)
_WFC_C = 1152             # rows 0:4, cols 1152:1216
_WFC2_C = 1216            # rows 0:64

_PROGRAM = None
LAST_RESULT = None


def _build_program():
    import concourse.bacc as bacc
    import concourse.mybir as mybir
    import concourse.tile as tile

    f32 = mybir.dt.float32
    f16 = mybir.dt.float16
    AF = mybir.ActivationFunctionType

    nc = bacc.Bacc()
    const_d = nc.dram_tensor("const", (128, CCOLS), f16, kind="ExternalInput")
    cbias_d = nc.dram_tensor("cbias", (128, 4), f32, kind="ExternalInput")
    init_d = nc.dram_tensor("init", (128, 128), f16, kind="ExternalInput")
    u0_d = nc.dram_tensor("u0", (NCHUNK, 128, 128), f16, kind="ExternalInput")
    hn_d = nc.dram_tensor("hn", (128, 128), f16, kind="ExternalOutput")
    out_d = nc.dram_tensor("out", (1, BC), f32, kind="ExternalOutput")

    with tile.TileContext(nc) as tc:
        with (
            tc.tile_pool(name="constp", bufs=1) as constp,
            tc.tile_pool(name="ringp", bufs=6) as ringp,
            tc.tile_pool(name="statep", bufs=4) as statep,
            tc.tile_pool(name="tailp", bufs=8) as tailp,
            tc.tile_pool(name="fcp", bufs=1) as fcp,
            tc.tile_pool(name="psump", bufs=4, space="PSUM") as psump,
            tc.tile_pool(name="psumfc", bufs=1, space="PSUM") as psumfc,
        ):
            ct = constp.tile([128, CCOLS], f16)
            cb = constp.tile([128, 4], f32)
            init = constp.tile([128, 128], f16)
            nc.gpsimd.dma_start(ct[:], const_d[:])
            nc.gpsimd.dma_start(cb[:], cbias_d[:])
            nc.gpsimd.dma_start(init[:], init_d[:])

            ring_tiles = []
            for j in range(NCHUNK):
                rt = ringp.tile([128, 128], f16, tag="ring", name=f"ring{j}")
                nc.gpsimd.dma_start(rt[:], u0_d[j])
                ring_tiles.append(rt)

            bias_ap = cb[:, 0:1]
            prev = init
            for s in range(NSTEPS):
                ps = psump.tile([128, 128], f32, tag="ps", name=f"ps{s}")
                if s < T:
                    m, par = (s % 8) // 2, s % 2
                    injc = 128 + m * 256 + par * 128
                    nc.tensor.matmul(
                        ps[:],
                        ct[32 * m:32 * m + 32, injc:injc + 128],
                        ring_tiles[s // 8][32 * m:32 * m + 32, :],
                        start=True, stop=False,
                        tile_position=(32 * m, 0),
                    )
                    nc.tensor.matmul(ps[:], ct[:, 0:128], prev[:],
                                     start=False, stop=True)
                else:
                    nc.tensor.matmul(ps[:], ct[:, 0:128], prev[:],
                                     start=True, stop=True)
                if s < L - 1:
                    # layers l > s must keep their initial hidden state:
                    # tanh writes the full tile, then an SBUF->SBUF DMA
                    # (no partition-alignment limit) overwrites rows with init
                    st = statep.tile([128, 128], f16, tag="st", name=f"st{s}")
                    nc.scalar.activation(st[:], ps[:], AF.Tanh, bias=bias_ap)
                    k = (s + 1) * 16
                    nc.gpsimd.dma_start(st[k:128, :], init[k:128, :])
                elif s >= T - 1:
                    st = tailp.tile([128, 128], f16, tag="tail", name=f"st{s}")
                    nc.scalar.activation(st[:], ps[:], AF.Tanh, bias=bias_ap)
                    lcap = s - (T - 1)
                    nc.gpsimd.dma_start(hn_d[16 * lcap:16 * lcap + 16, :],
                                        st[16 * lcap:16 * lcap + 16, :])
                else:
                    st = statep.tile([128, 128], f16, tag="st", name=f"st{s}")
                    nc.scalar.activation(st[:], ps[:], AF.Tanh, bias=bias_ap)
                prev = st

            # FC head on h_n[7] (= last tail tile rows 112:128)
            last = prev
            h7T = fcp.tile([4, BC], f16)
            for g in range(4):
                nc.gpsimd.dma_start(h7T[0:4, g * 128:(g + 1) * 128],
                                    last[112 + 4 * g:116 + 4 * g, :])
            psfc = psumfc.tile([64, BC], f32, tag="psfc")
            nc.tensor.matmul(psfc[:], ct[0:4, _WFC_C:_WFC_C + 64], h7T[:],
                             start=True, stop=True)
            hfc = fcp.tile([64, BC], f16)
            nc.scalar.activation(hfc[:], psfc[:], AF.Relu,
                                 bias=cb[0:64, 1:2])
            psfc2 = psumfc.tile([1, BC], f32, tag="psfc2")
            nc.tensor.matmul(psfc2[:], ct[0:64, _WFC2_C:_WFC2_C + 1], hfc[:],
                             start=True, stop=True)
            outsb = fcp.tile([1, BC], f32)
            nc.scalar.activation(outsb[:], psfc2[:], AF.Identity,
                                 bias=cb[0:1, 2:3])
            nc.gpsimd.dma_start(out_d[:], outsb[:])

    nc.finalize()
    return nc


def _get_program():
    global _PROGRAM
    if _PROGRAM is None:
        _PROGRAM = _build_program()
    return _PROGRAM


def _build_const(W_ih, W_hh, b_ih, b_hh, W_fc, b_fc, W_fc2, b_fc2):
    ct = np.zeros((128, CCOLS), np.float32)
    # S4: [p_in=(lam,g,k), p_out=(l,g,h')] block-structured RNN weights
    for l in range(L):
        for g in range(4):
            for hp in range(HID):
                p_out = l * 16 + g * 4 + hp
                for k in range(HID):
                    ct[l * 16 + g * 4 + k, p_out] = W_hh[l, hp, k]
                    if l >= 1:
                        ct[(l - 1) * 16 + g * 4 + k, p_out] = W_ih[l, hp, k]
    # inject stationaries I(m,par) [32,128] at rows 32m, cols 128+m*256+par*128
    for m in range(4):
        for par in range(2):
            c0 = 128 + m * 256 + par * 128
            for g in range(4):
                for hp in range(HID):
                    for k in range(HID):
                        ct[32 * m + par * 16 + g * 4 + k,
                           c0 + g * 4 + hp] = W_ih[0, hp, k]
    # FC head
    ct[0:4, _WFC_C:_WFC_C + 64] = W_fc.T          # [4, 64]
    ct[0:64, _WFC2_C] = W_fc2[0]                  # [64]
    cb = np.zeros((128, 4), np.float32)
    for l in range(L):
        for g in range(4):
            for h in range(HID):
                cb[l * 16 + g * 4 + h, 0] = b_ih[l, h] + b_hh[l, h]
    cb[0:64, 1] = b_fc
    cb[0, 2] = b_fc2[0]
    return ct.astype(np.float16), cb


def kernel(x, hidden, W_ih, W_hh, b_ih, b_hh, W_fc, b_fc, W_fc2, b_fc2):
    global LAST_RESULT
    from concourse.bass_utils import run_bass_kernel_spmd

    x = np.asarray(x, np.float32)
    hidden = np.asarray(hidden, np.float32)
    W_ih = np.asarray(W_ih, np.float32)
    W_hh = np.asarray(W_hh, np.float32)
    b_ih = np.asarray(b_ih, np.float32)
    b_hh = np.asarray(b_hh, np.float32)
    W_fc = np.asarray(W_fc, np.float32)
    b_fc = np.asarray(b_fc, np.float32)
    W_fc2 = np.asarray(W_fc2, np.float32)
    b_fc2 = np.asarray(b_fc2, np.float32)

    ct, cb = _build_const(W_ih, W_hh, b_ih, b_hh, W_fc, b_fc, W_fc2, b_fc2)

    # u0 relayout: x [B,T,4] -> per-core ring [NCHUNK, 128=(r,g,k), 128=b]
    u0 = x.reshape(NCORES, 4, 128, NCHUNK, 8, HID)          # c,g,b,j,r,k
    u0 = np.ascontiguousarray(
        u0.transpose(0, 3, 4, 1, 5, 2), dtype=np.float16)   # c,j,r,g,k,b
    u0 = u0.reshape(NCORES, NCHUNK, 128, 128)
    # init state: hidden [L,B,4] -> per-core [128=(l,g,h), 128=b]
    init = hidden.reshape(L, NCORES, 4, 128, HID)           # l,c,g,b,h
    init = np.ascontiguousarray(
        init.transpose(1, 0, 2, 4, 3), dtype=np.float16)    # c,l,g,h,b
    init = init.reshape(NCORES, 128, 128)

    nc = _get_program()
    in_maps = [{"const": ct, "cbias": cb, "init": init[c], "u0": u0[c]}
               for c in range(NCORES)]
    trace = bool(os.environ.get("BASS_TRACE"))
    res = run_bass_kernel_spmd(nc, in_maps, core_ids=list(range(NCORES)),
                               trace=trace)
    LAST_RESULT = res

    output = np.empty((B, 1), np.float32)
    h_n = np.empty((L, B, HID), np.float32)
    for c in range(NCORES):
        output[c * BC:(c + 1) * BC, 0] = res.results[c]["out"][0]
        hnc = res.results[c]["hn"].astype(np.float32)
        hnc = hnc.reshape(L, 4, HID, 128)                   # l,g,h,b
        h_n[:, c * BC:(c + 1) * BC, :] = (
            hnc.transpose(0, 1, 3, 2).reshape(L, BC, HID)
        )
    return output, h_n


# revision 5
# speedup vs baseline: 1.0712x; 1.0712x over previous
"""Trainium2 Bass kernel for the stacked tanh-RNN (BredictNetwork).

Strategy: data-parallel over batch (4096 -> 512/core on 8 cores). Per core a
*wavefront* schedule updates all 8 layers simultaneously each time step:
  H_s[l] = tanh(Wih_l @ H_{s-1}[l-1] + Whh_l @ H_{s-1}[l] + b_l),  H_s[-1]=x_s
State layout [128 part, 128 free]: partition p = l*16 + g*4 + h (l layer,
g batch-group of 128, h hidden), free = batch-within-group. One 128x128
fp16 matmul (block-structured weights, host-assembled, fp32 PSUM) + one
PSUM-accumulate inject matmul (the x drive, from a DMA'd ring of relayouted
x) + one tanh (per-partition fp32 bias) per step; 2048+7 steps. FC head
computed on-core at the end. h_n captured from the last 8 wavefront steps.
fp16 operand precision validated offline: out relerr 1.5e-4, hn 9.7e-4.
"""
import os
import numpy as np

T = 2048
L = 8
HID = 4
B = 4096
NCORES = 8
BC = B // NCORES          # 512 batch per core
NSTEPS = T + L - 1        # 2055
NCHUNK = T // 8           # 256 ring chunks (8 steps each)
CCOLS = 1280              # fp16 const block columns (padded)
_WFC_C = 1152             # rows 0:4, cols 1152:1216
_WFC2_C = 1216            # rows 0:64

_PROGRAM = None
LAST_RESULT = None


def _build_program():
    import concourse.bacc as bacc
    import concourse.mybir as mybir
    import concourse.tile as tile

    f32 = mybir.dt.float32
    f16 = mybir.dt.float16
    AF = mybir.ActivationFunctionType

    nc = bacc.Bacc()
    const_d = nc.dram_tensor("const", (128, CCOLS), f16, kind="ExternalInput")
    cbias_d = nc.dram_tensor("cbias", (128, 4), f32, kind="ExternalInput")
    init_d = nc.dram_tensor("init", (128, 128), f16, kind="ExternalInput")
    u0_d = nc.dram_tensor("u0", (NCHUNK, 128, 128), f16, kind="ExternalInput")
    hn_d = nc.dram_tensor("hn", (128, 128), f16, kind="ExternalOutput")
    out_d = nc.dram_tensor("out", (1, BC), f32, kind="ExternalOutput")

    with tile.TileContext(nc) as tc:
        with (
            tc.tile_pool(name="constp", bufs=1) as constp,
            tc.tile_pool(name="ringp", bufs=6) as ringp,
            tc.tile_pool(name="statep", bufs=4) as statep,
            tc.tile_pool(name="tailp", bufs=8) as tailp,
            tc.tile_pool(name="fcp", bufs=1) as fcp,
            tc.tile_pool(name="psump", bufs=4, space="PSUM") as psump,
            tc.tile_pool(name="psumfc", bufs=1, space="PSUM") as psumfc,
        ):
            ct = constp.tile([128, CCOLS], f16)
            cb = constp.tile([128, 4], f32)
            init = constp.tile([128, 128], f16)
            nc.gpsimd.dma_start(ct[:], const_d[:])
            nc.gpsimd.dma_start(cb[:], cbias_d[:])
            nc.gpsimd.dma_start(init[:], init_d[:])

            ring_tiles = []
            for j in range(NCHUNK):
                rt = ringp.tile([128, 128], f16, tag="ring", name=f"ring{j}")
                nc.gpsimd.dma_start(rt[:], u0_d[j])
                ring_tiles.append(rt)

            bias_ap = cb[:, 0:1]
            prev = init
            for s in range(NSTEPS):
                ps = psump.tile([128, 128], f32, tag="ps", name=f"ps{s}")
                if s < T:
                    m, par = (s % 8) // 2, s % 2
                    injc = 128 + m * 256 + par * 128
                    nc.tensor.matmul(
                        ps[:],
                        ct[32 * m:32 * m + 32, injc:injc + 128],
                        ring_tiles[s // 8][32 * m:32 * m + 32, :],
                        start=True, stop=False,
                        tile_position=(32 * m, 0),
                    )
                    nc.tensor.matmul(ps[:], ct[:, 0:128], prev[:],
                                     start=False, stop=True)
                else:
                    nc.tensor.matmul(ps[:], ct[:, 0:128], prev[:],
                                     start=True, stop=True)
                if s < L - 1:
                    # layers l > s must keep their initial hidden state:
                    # tanh writes the full tile, then an SBUF->SBUF DMA
                    # (no partition-alignment limit) overwrites rows with init
                    st = statep.tile([128, 128], f16, tag="st", name=f"st{s}")
                    nc.scalar.activation(st[:], ps[:], AF.Tanh, bias=bias_ap)
                    k = (s + 1) * 16
                    nc.gpsimd.dma_start(st[k:128, :], init[k:128, :])
                elif s >= T - 1:
                    st = tailp.tile([128, 128], f16, tag="tail", name=f"st{s}")
                    nc.scalar.activation(st[:], ps[:], AF.Tanh, bias=bias_ap)
                    lcap = s - (T - 1)
                    nc.gpsimd.dma_start(hn_d[16 * lcap:16 * lcap + 16, :],
                                        st[16 * lcap:16 * lcap + 16, :])
                else:
                    st = statep.tile([128, 128], f16, tag="st", name=f"st{s}")
                    nc.scalar.activation(st[:], ps[:], AF.Tanh, bias=bias_ap)
                prev = st

            # FC head on h_n[7] (= last tail tile rows 112:128)
            last = prev
            h7T = fcp.tile([4, BC], f16)
            for g in range(4):
                nc.gpsimd.dma_start(h7T[0:4, g * 128:(g + 1) * 128],
                                    last[112 + 4 * g:116 + 4 * g, :])
            psfc = psumfc.tile([64, BC], f32, tag="psfc")
            nc.tensor.matmul(psfc[:], ct[0:4, _WFC_C:_WFC_C + 64], h7T[:],
                             start=True, stop=True)
            hfc = fcp.tile([64, BC], f16)
            nc.scalar.activation(hfc[:], psfc[:], AF.Relu,
                                 bias=cb[0:64, 1:2])
            psfc2 = psumfc.tile([1, BC], f32, tag="psfc2")
            nc.tensor.matmul(psfc2[:], ct[0:64, _WFC2_C:_WFC2_C + 1], hfc[:],
                             start=True, stop=True)
            outsb = fcp.tile([1, BC], f32)
            nc.scalar.activation(outsb[:], psfc2[:], AF.Identity,
                                 bias=cb[0:1, 2:3])
            nc.gpsimd.dma_start(out_d[:], outsb[:])

    # Strip same-engine semaphore waits (WAW buffer-recycle deps): each
    # engine's queue is strict in-order FIFO, so a wait on the engine's own
    # semaphore is redundant — and removing it lets Bacc skip the
    # EventSemaphore split (2 waits -> 1), removing a hop from the serial
    # chain. PE is left untouched (it has a LDWEIGHTS reorder window).
    _SELF = {
        "Activation": "Activation_",
        "DVE": "Vector_",
        "Pool": "Pool_",
    }
    import concourse.mybir as _mybir
    for blk in nc.m.functions[0].blocks:
        for inst in blk.instructions:
            eng = str(getattr(inst, "engine", "")).replace("EngineType.", "")
            pre = _SELF.get(eng)
            si = inst.sync_info
            if pre is None or si is None or not si.on_wait:
                continue
            kept = [w for w in si.on_wait
                    if w.wait_reg is not None or not w.ant_name.startswith(pre)]
            if len(kept) != len(si.on_wait):
                inst.sync_info = _mybir.SyncInfo(on_wait=kept,
                                                 on_update=si.on_update)

    nc.finalize()
    return nc


def _get_program():
    global _PROGRAM
    if _PROGRAM is None:
        _PROGRAM = _build_program()
    return _PROGRAM


def _build_const(W_ih, W_hh, b_ih, b_hh, W_fc, b_fc, W_fc2, b_fc2):
    ct = np.zeros((128, CCOLS), np.float32)
    # S4: [p_in=(lam,g,k), p_out=(l,g,h')] block-structured RNN weights
    for l in range(L):
        for g in range(4):
            for hp in range(HID):
                p_out = l * 16 + g * 4 + hp
                for k in range(HID):
                    ct[l * 16 + g * 4 + k, p_out] = W_hh[l, hp, k]
                    if l >= 1:
                        ct[(l - 1) * 16 + g * 4 + k, p_out] = W_ih[l, hp, k]
    # inject stationaries I(m,par) [32,128] at rows 32m, cols 128+m*256+par*128
    for m in range(4):
        for par in range(2):
            c0 = 128 + m * 256 + par * 128
            for g in range(4):
                for hp in range(HID):
                    for k in range(HID):
                        ct[32 * m + par * 16 + g * 4 + k,
                           c0 + g * 4 + hp] = W_ih[0, hp, k]
    # FC head
    ct[0:4, _WFC_C:_WFC_C + 64] = W_fc.T          # [4, 64]
    ct[0:64, _WFC2_C] = W_fc2[0]                  # [64]
    cb = np.zeros((128, 4), np.float32)
    for l in range(L):
        for g in range(4):
            for h in range(HID):
                cb[l * 16 + g * 4 + h, 0] = b_ih[l, h] + b_hh[l, h]
    cb[0:64, 1] = b_fc
    cb[0, 2] = b_fc2[0]
    return ct.astype(np.float16), cb


def kernel(x, hidden, W_ih, W_hh, b_ih, b_hh, W_fc, b_fc, W_fc2, b_fc2):
    global LAST_RESULT
    from concourse.bass_utils import run_bass_kernel_spmd

    x = np.asarray(x, np.float32)
    hidden = np.asarray(hidden, np.float32)
    W_ih = np.asarray(W_ih, np.float32)
    W_hh = np.asarray(W_hh, np.float32)
    b_ih = np.asarray(b_ih, np.float32)
    b_hh = np.asarray(b_hh, np.float32)
    W_fc = np.asarray(W_fc, np.float32)
    b_fc = np.asarray(b_fc, np.float32)
    W_fc2 = np.asarray(W_fc2, np.float32)
    b_fc2 = np.asarray(b_fc2, np.float32)

    ct, cb = _build_const(W_ih, W_hh, b_ih, b_hh, W_fc, b_fc, W_fc2, b_fc2)

    # u0 relayout: x [B,T,4] -> per-core ring [NCHUNK, 128=(r,g,k), 128=b]
    u0 = x.reshape(NCORES, 4, 128, NCHUNK, 8, HID)          # c,g,b,j,r,k
    u0 = np.ascontiguousarray(
        u0.transpose(0, 3, 4, 1, 5, 2), dtype=np.float16)   # c,j,r,g,k,b
    u0 = u0.reshape(NCORES, NCHUNK, 128, 128)
    # init state: hidden [L,B,4] -> per-core [128=(l,g,h), 128=b]
    init = hidden.reshape(L, NCORES, 4, 128, HID)           # l,c,g,b,h
    init = np.ascontiguousarray(
        init.transpose(1, 0, 2, 4, 3), dtype=np.float16)    # c,l,g,h,b
    init = init.reshape(NCORES, 128, 128)

    nc = _get_program()
    in_maps = [{"const": ct, "cbias": cb, "init": init[c], "u0": u0[c]}
               for c in range(NCORES)]
    trace = bool(os.environ.get("BASS_TRACE"))
    res = run_bass_kernel_spmd(nc, in_maps, core_ids=list(range(NCORES)),
                               trace=trace)
    LAST_RESULT = res

    output = np.empty((B, 1), np.float32)
    h_n = np.empty((L, B, HID), np.float32)
    for c in range(NCORES):
        output[c * BC:(c + 1) * BC, 0] = res.results[c]["out"][0]
        hnc = res.results[c]["hn"].astype(np.float32)
        hnc = hnc.reshape(L, 4, HID, 128)                   # l,g,h,b
        h_n[:, c * BC:(c + 1) * BC, :] = (
            hnc.transpose(0, 1, 3, 2).reshape(L, BC, HID)
        )
    return output, h_n
